# revision 11
# baseline (speedup 1.0000x reference)
"""Binary linear layer (sign(X) @ sign(W) * scale) on 8 trn2 NeuronCores.

Strategy: data-parallel over the batch dim. Each core gets 1/8 of X's rows
(host-transposed + k-subtile-reordered so one DMA descriptor covers any
k-range), the full W, and alpha/betta/gamma. Uploads are fp8e5 with a x2^10
host pre-scale (sign-exact; fp8e5 zero-rounding probability ~2e-7). On-device:
binarization of X on the DVE (+-0.5, with the compensating 2x folded into
relu(alpha)) and of W on the ACT engine (+-1), fp8 DoubleRow matmuls on the
PE, single-phase PSUM accumulation over the whole K with nt-outer waves
(8 PSUM banks = the 8 m-tiles of a wave), fused scale multiply on the DVE,
f16 output (host-cast to f32).
"""

import os

import numpy as np

import concourse.bacc as bacc
import concourse.mybir as mybir
from concourse.tile import TileContext

P = 128
B, IN, OUT = 8192, 4096, 2048
NCORES = 8


def build_kernel(
    M=B // NCORES,
    K=IN,
    N=OUT,
    fd=512,  # matmul moving free dim (psum tile width); DR max is 512
    kpd=2,  # kpairs per staging DMA descriptor (issue cost amortization)
    stx_bufs=3,
    stw_bufs=8,
    ot_bufs=8,
    warmup_mms=10,  # junk PE matmuls at t=0 to lift the HAM clock gate
    out_dt=mybir.dt.float16,
):
    f32 = mybir.dt.float32
    bin_dt = mybir.dt.float8e4
    up_dt = mybir.dt.float8e5
    kp = 2  # k-subtiles per DoubleRow matmul
    pm = mybir.MatmulPerfMode.DoubleRow

    ksub = K // P
    nkp = ksub // kp
    nmt = M // P
    nnc = N // fd
    assert K % (P * kp) == 0 and M % P == 0 and N % fd == 0
    assert nkp % kpd == 0
    assert nmt <= 8  # one PSUM bank per m-tile within a wave

    nc = bacc.Bacc("TRN2", debug=False, num_devices=NCORES)

    # host layouts: XT[p, s, m] = X.T[s*128+p, m], W[p, s, n] = W[s*128+p, n]
    XT = nc.declare_dram_parameter("XT", [P, ksub, M], up_dt, isOutput=False)
    W = nc.declare_dram_parameter("W", [P, ksub, N], up_dt, isOutput=False)
    alpha = nc.declare_dram_parameter("alpha", [1], f32, isOutput=False)
    betta = nc.declare_dram_parameter("betta", [32], f32, isOutput=False)
    gamma = nc.declare_dram_parameter("gamma", [64], f32, isOutput=False)
    Y = nc.declare_dram_parameter("Y", [M, N], out_dt, isOutput=True)

    AF = mybir.ActivationFunctionType

    with TileContext(nc) as tc:
        with (
            tc.tile_pool(name="const", bufs=1) as cpool,
            tc.tile_pool(name="stage", bufs=2) as stpool,
            tc.tile_pool(name="bin", bufs=3) as binpool,
            tc.tile_pool(name="outp", bufs=4) as outpool,
            tc.tile_pool(name="psum", bufs=8, space="PSUM") as pspool,
        ):
            # ---- PE warm-up: fills the otherwise-idle pipeline-fill window
            # with junk matmuls so the HAM un-throttles before real MMs ----
            if warmup_mms:
                wu = cpool.tile([P, kp, fd // 2], bin_dt, bufs=1)
                nc.vector.memset(wu, 0)
                ps_wu = pspool.tile([P, fd], f32, tag="mm", bufs=8)
                for _ in range(warmup_mms):
                    nc.tensor.matmul(
                        ps_wu[:, : fd // 2],
                        lhsT=wu[:, :, :P],
                        rhs=wu,
                        start=True,
                        stop=True,
                        perf_mode=pm,
                    )

            # ---- scale grid: 2*relu(alpha) * outer(relu(betta), relu(gamma))
            # (the 2x compensates X's +-0.5 encoding), partition-broadcast ----
            scale_t = cpool.tile([P, N], f32, bufs=1)
            a_t = cpool.tile([1, 1], f32, bufs=1)
            b_t = cpool.tile([1, 32], f32, bufs=1)
            g_t = cpool.tile([1, 64], f32, bufs=1)
            nc.gpsimd.dma_start(out=a_t, in_=alpha[:])
            nc.gpsimd.dma_start(out=b_t, in_=betta[:])
            nc.gpsimd.dma_start(out=g_t, in_=gamma[:])
            # relus on the gpsimd queue (same as the a/b/g DMAs) so the ACT
            # engine's FIFO stays clear for the W binarize stream
            nc.gpsimd.tensor_scalar(
                out=a_t,
                in0=a_t,
                scalar1=0.0,
                scalar2=2.0,
                op0=mybir.AluOpType.max,
                op1=mybir.AluOpType.mult,
            )
            # relu(betta)*relu(alpha) == relu(betta*relu(alpha)): relu(alpha)>=0
            b_s = cpool.tile([1, 32], f32, bufs=1)
            g_r = cpool.tile([1, 64], f32, bufs=1)
            nc.gpsimd.tensor_scalar(
                out=b_s,
                in0=b_t,
                scalar1=0.0,
                scalar2=a_t[0:1, 0:1],
                op0=mybir.AluOpType.max,
                op1=mybir.AluOpType.mult,
            )
            nc.gpsimd.tensor_scalar_max(out=g_r, in0=g_t, scalar1=0.0)
            ps_sc = pspool.tile([P, fd], f32, tag="mm", bufs=8)
            nc.tensor.matmul(
                ps_sc[:32, :64], lhsT=b_s, rhs=g_r, start=True, stop=True
            )
            # flatten [32,64] -> row 0, then log-doubling partition broadcast
            sc_tmp = cpool.tile([32, 64], f32, bufs=1)
            nc.scalar.copy(sc_tmp, ps_sc[:32, :64])
            nc.gpsimd.dma_start(out=scale_t[0:1, :], in_=sc_tmp)
            sh = 1
            while sh < P:
                nc.gpsimd.dma_start(
                    out=scale_t[sh : 2 * sh, :], in_=scale_t[0:sh, :]
                )
                sh *= 2
            for w in range(1, nnc, 2):
                # odd waves' W uses +-0.5 encoding -> 2x their columns
                nc.gpsimd.tensor_scalar_mul(
                    out=scale_t[:, w * fd : (w + 1) * fd],
                    in0=scale_t[:, w * fd : (w + 1) * fd],
                    scalar1=2.0,
                )

            def emit_wstage(w, cg):
                """Stage kpd kpairs of wave w's W column block; binarize per
                kpair on ACT (+-1) for even waves, DVE (+-0.5, with the 2x
                folded into that wave's scale columns) for odd waves."""
                ws = stpool.tile(
                    [P, kp * kpd, fd], up_dt, tag="ws", bufs=stw_bufs
                )
                nc.sync.dma_start(
                    out=ws,
                    in_=W[
                        :,
                        cg * kp * kpd : (cg + 1) * kp * kpd,
                        w * fd : (w + 1) * fd,
                    ],
                )
                wb = binpool.tile(
                    [P, kp * kpd, fd], bin_dt, tag="wb", bufs=2 * nkp // kpd
                )
                outs = []
                for j in range(kpd):
                    if w % 2 == 0:
                        nc.scalar.activation(
                            wb[:, j * kp : (j + 1) * kp, :],
                            ws[:, j * kp : (j + 1) * kp, :],
                            AF.Sign,
                        )
                    else:
                        nc.vector.tensor_scalar(
                            out=wb[:, j * kp : (j + 1) * kp, :],
                            in0=ws[:, j * kp : (j + 1) * kp, :],
                            scalar1=0.0,
                            scalar2=0.5,
                            op0=mybir.AluOpType.is_ge,
                            op1=mybir.AluOpType.subtract,
                        )
                    outs.append(wb[:, j * kp : (j + 1) * kp, :])
                return outs

            # ---- prologue: stage+binarize all of X and wave 0's W ----
            xbs = []
            wbs = []
            for cg in range(nkp // kpd):
                xs = stpool.tile(
                    [P, kp * kpd, M], up_dt, tag="xs", bufs=stx_bufs
                )
                nc.sync.dma_start(
                    out=xs, in_=XT[:, cg * kp * kpd : (cg + 1) * kp * kpd, :]
                )
                xb = binpool.tile(
                    [P, kp * kpd, M], bin_dt, tag="xb", bufs=nkp // kpd
                )
                for j in range(kpd):
                    # +-0.5 on the DVE; the 2x is folded into relu(alpha)
                    nc.vector.tensor_scalar(
                        out=xb[:, j * kp : (j + 1) * kp, :],
                        in0=xs[:, j * kp : (j + 1) * kp, :],
                        scalar1=0.0,
                        scalar2=0.5,
                        op0=mybir.AluOpType.is_ge,
                        op1=mybir.AluOpType.subtract,
                    )
                    xbs.append(xb[:, j * kp : (j + 1) * kp, :])
                wbs.extend(emit_wstage(0, cg))

            # ---- nt-outer waves; 8 PSUM banks = the 8 m-tiles of a wave.
            # Evicts copy RAW sums psum->SBUF f16 (exact: sums are multiples
            # of 0.25 well inside f16's exact range), freeing the PSUM bank
            # without touching scale_t. The scale multiply + output DMA run
            # ~2 waves later, so the slow scale broadcast chain never gates
            # the matmul stream.
            raws = {}

            def emit_copy(w, ps_mt, mt):
                raw = outpool.tile([P, fd], out_dt, tag="raw", bufs=nnc * nmt)
                nc.vector.tensor_copy(out=raw, in_=ps_mt)
                raws[(w, mt)] = raw

            def emit_finish(w, mt):
                raw = raws[(w, mt)]
                nc.vector.tensor_mul(
                    out=raw, in0=raw, in1=scale_t[:, w * fd : (w + 1) * fd]
                )
                nc.gpsimd.dma_start(
                    out=Y[mt * P : (mt + 1) * P, w * fd : (w + 1) * fd],
                    in_=raw,
                )

            for w in range(nnc):
                ps = []
                for _mt in range(nmt):
                    ps_t = pspool.tile([P, fd], f32, tag="mm", bufs=8)
                    ps.append(ps_t)

                wbs_next = []
                if w < nnc - 1:
                    for c in range(nkp):
                        for mt in range(nmt):
                            nc.tensor.matmul(
                                ps[mt],
                                lhsT=xbs[c][:, :, mt * P : (mt + 1) * P],
                                rhs=wbs[c],
                                start=(c == 0),
                                stop=(c == nkp - 1),
                                perf_mode=pm,
                            )
                            if c == nkp - 1:
                                emit_copy(w, ps[mt], mt)
                        # next wave's W blocks ride behind this wave's MMs
                        if c % kpd == kpd - 1:
                            wbs_next.extend(emit_wstage(w + 1, c // kpd))
                        # wave w-2's scale-mul + output, spread mid-wave
                        if w >= 2 and 4 <= c < 4 + nmt:
                            emit_finish(w - 2, c - 4)
                else:
                    # final wave: mt-outer; each m-tile's copy+finish overlap
                    # the next m-tile's matmuls, and the waves-1/2 backlog is
                    # front-loaded so only mt7's copy+finish trail the last MM
                    for mt in range(nmt):
                        for c in range(nkp):
                            nc.tensor.matmul(
                                ps[mt],
                                lhsT=xbs[c][:, :, mt * P : (mt + 1) * P],
                                rhs=wbs[c],
                                start=(c == 0),
                                stop=(c == nkp - 1),
                                perf_mode=pm,
                            )
                        emit_copy(w, ps[mt], mt)
                        emit_finish(w, mt)
                        if mt == 0:
                            for m in range(nmt):
                                emit_finish(w - 2, m)
                                emit_finish(w - 1, m)
                wbs = wbs_next
    return nc


_NC_CACHE = {}


def _get_nc(**kw):
    key = tuple(sorted(kw.items()))
    if key not in _NC_CACHE:
        nc = build_kernel(**kw)
        nc.finalize()  # runs the bacc passes (reg alloc etc.) pre-serialization
        _NC_CACHE[key] = nc
    return _NC_CACHE[key]


def _make_in_maps(X, W, alpha, betta, gamma):
    f8 = mybir.dt.np(mybir.dt.float8e5)
    ksub = IN // P
    # x2^10 pre-scale is exact for sign() and pushes values clear of the
    # fp8e5 subnormal floor (rounds-to-zero prob ~2e-7 instead of ~1e-5)
    Ws = np.asarray(W, dtype=np.float32) * 1024.0
    Wr = np.ascontiguousarray(
        Ws.reshape(ksub, P, OUT).transpose(1, 0, 2).astype(f8)
    )
    alpha = np.asarray(alpha, dtype=np.float32).reshape([1])
    betta = np.asarray(betta, dtype=np.float32).reshape([32])
    gamma = np.asarray(gamma, dtype=np.float32).reshape([64])
    X = np.asarray(X, dtype=np.float32)
    rows = X.shape[0] // NCORES
    in_maps = []
    for c in range(NCORES):
        xt = (X[c * rows : (c + 1) * rows, :].T * 1024.0).reshape(
            ksub, P, rows
        )
        xt = np.ascontiguousarray(xt.transpose(1, 0, 2).astype(f8))
        in_maps.append(
            {"XT": xt, "W": Wr, "alpha": alpha, "betta": betta, "gamma": gamma}
        )
    return in_maps


def run_on_cores(inputs, trace=False, tmpdir=None, **build_kw):
    """Run the SPMD kernel on 8 cores; returns (Y_full, BassKernelResults)."""
    from concourse.bass_utils import run_bass_kernel_spmd

    if not trace:
        # this image lacks antenv.axon_hooks; a stray BASS_TRACE env var would
        # crash run_bass_kernel_spmd's trace branch, so fail safe
        try:
            import antenv.axon_hooks  # noqa: F401
        except ImportError:
            os.environ.setdefault("BASS_NEVER_TRACE", "1")
    nc = _get_nc(**build_kw)
    in_maps = _make_in_maps(**inputs)
    res = run_bass_kernel_spmd(
        nc, in_maps, list(range(NCORES)), trace=trace, tmpdir=tmpdir
    )
    Y = np.concatenate(
        [np.asarray(r["Y"], dtype=np.float32) for r in res.results], axis=0
    )
    return Y, res


PROD_KW = dict(
    fd=512,
    kpd=2,
    stx_bufs=3,
    stw_bufs=8,
    ot_bufs=8,
    warmup_mms=10,
)


def kernel(**inputs) -> np.ndarray:
    Y, _ = run_on_cores(inputs, **PROD_KW)
    return Y
